# revision 4
# baseline (speedup 1.0000x reference)
"""Conditional 1x1 conv (per-sample class-routed weights) on 8 Trainium2 cores.

Strategy (hardcoded for x:[32,64,64,512] f32, cls:[32,1] int64,
kernel:[120,1,1,512,512] f32, bias:[120,512] f32):

- Host: gather per-sample weight [B,C,F] = kernel[cls], transpose x to
  [B, C, HW] (channels-on-partitions layout so the device needs no
  transposes at all), cast x/w to bf16, shard batch 4-samples-per-core
  across 8 cores.
- Device (per core, SPMD): for each sample, per 128-row pixel tile,
  out[p,f] = sum_k xT[c,p].T @ w[c,f] accumulated over 4 c-chunks in PSUM
  (fp32), evacuated PSUM->SBUF as bf16 (alternating ACT/DVE) -> DRAM.
  bf16 in/out halves HBM traffic vs fp32 (~35MB/core at ~360GB/s) so the
  kernel runs at the tensor-engine roofline (~109us/core of matmul).
  All loads go through the Pool-engine SWDGE path and stores are batched
  per 512-pixel block on the SP HWDGE queue, keeping per-DMA fixed costs
  (632ns HWDGE serialization each) off the critical path.
- Host: concat core outputs, upcast to fp32, reshape back to [B,H,W,F].
"""

import numpy as np
import ml_dtypes

import concourse.bacc as bacc
import concourse.mybir as mybir
import concourse.tile as tile
from concourse import bass
from concourse.bass_utils import run_bass_kernel_spmd

B, H, W, C, F = 32, 64, 64, 512, 512
NCORES = 8
SPC = B // NCORES          # samples per core
NPIX = H * W               # 4096 pixels per sample
P = 128                    # partitions
KO = C // P                # 4 contraction chunks
PB = 512                   # pixel block per x-tile DMA / out-store batch
NPB = NPIX // PB           # 8 pixel blocks per sample
PT = PB // P               # 4 pixel tiles (matmul groups) per block

BF16 = ml_dtypes.bfloat16

_CACHE: dict = {}
_last_results = None       # test harness introspection


def _build(add_bias: bool, reps: int = 1):
    nc = bacc.Bacc("TRN2", target_bir_lowering=False, debug=False)
    xt_d = nc.declare_dram_parameter("xt", [SPC, C, NPIX], mybir.dt.bfloat16, isOutput=False)
    wt_d = nc.declare_dram_parameter("wt", [SPC, C, F], mybir.dt.bfloat16, isOutput=False)
    if add_bias:
        bt_d = nc.declare_dram_parameter("bt", [SPC, P, F], mybir.dt.float32, isOutput=False)
    out_d = nc.declare_dram_parameter("out", [SPC, NPIX, F], mybir.dt.bfloat16, isOutput=True)

    with tile.TileContext(nc) as tc:
        with (
            tc.tile_pool(name="xpool", bufs=8) as xpool,
            tc.tile_pool(name="wpool", bufs=2) as wpool,
            tc.tile_pool(name="opool", bufs=3) as opool,
            tc.tile_pool(name="pspool", bufs=8, space="PSUM") as pspool,
        ):
          for _rep in range(reps):
            for s in range(SPC):
                w_sb = wpool.tile([P, KO, F], mybir.dt.bfloat16, tag="w")
                nc.gpsimd.dma_start(
                    w_sb[:], wt_d[s].rearrange("(ko ki) f -> ki ko f", ki=P)
                )
                if add_bias:
                    b_sb = wpool.tile([P, F], mybir.dt.float32, tag="b")
                    nc.gpsimd.dma_start(b_sb[:], bt_d[s])
                for pb in range(NPB):
                    x_sb = xpool.tile([P, KO, PB], mybir.dt.bfloat16, tag="x")
                    nc.gpsimd.dma_start(
                        x_sb[:],
                        xt_d[s].rearrange("(ko ki) p -> ki ko p", ki=P)[
                            :, :, pb * PB : (pb + 1) * PB
                        ],
                    )
                    o_sb = opool.tile([P, PT, F], mybir.dt.bfloat16, tag="o")
                    for j in range(PT):
                        ps = pspool.tile([P, F], mybir.dt.float32, tag="ps")
                        for k in range(KO):
                            nc.tensor.matmul(
                                ps[:],
                                x_sb[:, k, bass.ts(j, P)],
                                w_sb[:, k, :],
                                start=(k == 0),
                                stop=(k == KO - 1),
                            )
                        if add_bias:
                            nc.vector.tensor_tensor(
                                o_sb[:, j, :], ps[:], b_sb[:], mybir.AluOpType.add
                            )
                        elif j % 2 == 0:
                            nc.scalar.copy(out=o_sb[:, j, :], in_=ps[:])
                        else:
                            nc.vector.tensor_copy(out=o_sb[:, j, :], in_=ps[:])
                    row0 = pb * PB
                    nc.sync.dma_start(
                        out_d[s, row0 : row0 + PB, :].rearrange(
                            "(pt pi) f -> pi pt f", pi=P
                        ),
                        o_sb[:],
                    )
    nc.compile()
    return nc


def kernel(x, cls, kernel, bias):
    global _last_results
    x = np.asarray(x, dtype=np.float32)
    cls_idx = np.asarray(cls).reshape(-1).astype(np.int64)
    ktab = np.asarray(kernel, dtype=np.float32).reshape(-1, C, F)
    bias = np.asarray(bias, dtype=np.float32)

    # host-side routing + layout prep
    w_all = ktab[cls_idx].astype(BF16)                      # [B, C, F]
    b_all = bias[cls_idx]                                   # [B, F]
    add_bias = bool(np.any(b_all))
    xt_all = np.ascontiguousarray(
        x.reshape(B, NPIX, C).transpose(0, 2, 1)            # [B, C, NPIX]
    ).astype(BF16)

    key = ("cc11", add_bias)
    if key not in _CACHE:
        _CACHE[key] = _build(add_bias)
    nc = _CACHE[key]

    in_maps = []
    for c in range(NCORES):
        sl = slice(c * SPC, (c + 1) * SPC)
        m = {
            "xt": np.ascontiguousarray(xt_all[sl]),
            "wt": np.ascontiguousarray(w_all[sl]),
        }
        if add_bias:
            m["bt"] = np.ascontiguousarray(
                np.broadcast_to(b_all[sl, None, :], (SPC, P, F))
            )
        in_maps.append(m)

    res = run_bass_kernel_spmd(nc, in_maps, list(range(NCORES)))
    _last_results = res

    out = np.concatenate([res.results[c]["out"] for c in range(NCORES)], axis=0)
    return out.astype(np.float32).reshape(B, H, W, F)


# revision 5
# speedup vs baseline: 1.2962x; 1.2962x over previous
"""Conditional 1x1 conv (per-sample class-routed weights) on 8 Trainium2 cores.

Strategy (hardcoded for x:[32,64,64,512] f32, cls:[32,1] int64,
kernel:[120,1,1,512,512] f32, bias:[120,512] f32):

- Host: gather per-sample weight [B,C,F] = kernel[cls], transpose x to
  [B, C, HW] (channels-on-partitions layout so the device needs no
  transposes at all), cast x/w to bf16, shard batch 4-samples-per-core
  across 8 cores.
- Device (per core, SPMD): for each sample, per 128-row pixel tile,
  out[p,f] = sum_k xT[c,p].T @ w[c,f] accumulated over 4 c-chunks in PSUM
  (fp32), evacuated PSUM->SBUF as bf16 (alternating ACT/DVE) -> DRAM.
  bf16 in/out halves HBM traffic vs fp32 (~35MB/core at ~360GB/s) so the
  kernel runs at the tensor-engine roofline (~109us/core of matmul).
  All loads go through the Pool-engine SWDGE path and stores are batched
  per 512-pixel block on the SP HWDGE queue, keeping per-DMA fixed costs
  (632ns HWDGE serialization each) off the critical path.
- Host: concat core outputs, upcast to fp32, reshape back to [B,H,W,F].
"""

import numpy as np
import ml_dtypes

import concourse.bacc as bacc
import concourse.mybir as mybir
import concourse.tile as tile
from concourse import bass
from concourse.bass_utils import run_bass_kernel_spmd

B, H, W, C, F = 32, 64, 64, 512, 512
NCORES = 8
SPC = B // NCORES          # samples per core
NPIX = H * W               # 4096 pixels per sample
P = 128                    # partitions
KO = C // P                # 4 contraction chunks
PB = 512                   # pixel block per x-tile DMA / out-store batch
NPB = NPIX // PB           # 8 pixel blocks per sample
PT = PB // P               # 4 pixel tiles (matmul groups) per block

BF16 = ml_dtypes.bfloat16

_CACHE: dict = {}
_last_results = None       # test harness introspection


def _build(add_bias: bool, reps: int = 1):
    nc = bacc.Bacc("TRN2", target_bir_lowering=False, debug=False)
    xt_d = nc.declare_dram_parameter("xt", [SPC, C, NPIX], mybir.dt.bfloat16, isOutput=False)
    wt_d = nc.declare_dram_parameter("wt", [SPC, C, F], mybir.dt.bfloat16, isOutput=False)
    if add_bias:
        bt_d = nc.declare_dram_parameter("bt", [SPC, P, F], mybir.dt.float32, isOutput=False)
    # out is stored pixel-tiled [s, pb, pi, pt, f] (pixel = pb*PB + pt*P + pi)
    # so each store is one fully contiguous 512 KiB block (4 KiB per
    # partition row -> 4x fewer DMA descriptors than the pixel-major
    # layout); the host un-tiles afterwards.
    out_d = nc.declare_dram_parameter("out", [SPC, NPB, P, PT, F], mybir.dt.bfloat16, isOutput=True)

    with tile.TileContext(nc) as tc:
        with (
            tc.tile_pool(name="xpool", bufs=8) as xpool,
            tc.tile_pool(name="wpool", bufs=2) as wpool,
            tc.tile_pool(name="opool", bufs=3) as opool,
            tc.tile_pool(name="pspool", bufs=8, space="PSUM") as pspool,
        ):
          for _rep in range(reps):
            for s in range(SPC):
                w_sb = wpool.tile([P, KO, F], mybir.dt.bfloat16, tag="w")
                nc.gpsimd.dma_start(
                    w_sb[:], wt_d[s].rearrange("(ko ki) f -> ki ko f", ki=P)
                )
                if add_bias:
                    b_sb = wpool.tile([P, F], mybir.dt.float32, tag="b")
                    nc.gpsimd.dma_start(b_sb[:], bt_d[s])
                for pb in range(NPB):
                    x_sb = xpool.tile([P, KO, PB], mybir.dt.bfloat16, tag="x")
                    nc.gpsimd.dma_start(
                        x_sb[:],
                        xt_d[s].rearrange("(ko ki) p -> ki ko p", ki=P)[
                            :, :, pb * PB : (pb + 1) * PB
                        ],
                    )
                    o_sb = opool.tile([P, PT, F], mybir.dt.bfloat16, tag="o")
                    for j in range(PT):
                        ps = pspool.tile([P, F], mybir.dt.float32, tag="ps")
                        for k in range(KO):
                            nc.tensor.matmul(
                                ps[:],
                                x_sb[:, k, bass.ts(j, P)],
                                w_sb[:, k, :],
                                start=(k == 0),
                                stop=(k == KO - 1),
                            )
                        if add_bias:
                            nc.vector.tensor_tensor(
                                o_sb[:, j, :], ps[:], b_sb[:], mybir.AluOpType.add
                            )
                        elif j % 2 == 0:
                            nc.scalar.copy(out=o_sb[:, j, :], in_=ps[:])
                        else:
                            nc.vector.tensor_copy(out=o_sb[:, j, :], in_=ps[:])
                    nc.sync.dma_start(out_d[s, pb], o_sb[:])
    nc.compile()
    return nc


def kernel(x, cls, kernel, bias):
    global _last_results
    x = np.asarray(x, dtype=np.float32)
    cls_idx = np.asarray(cls).reshape(-1).astype(np.int64)
    ktab = np.asarray(kernel, dtype=np.float32).reshape(-1, C, F)
    bias = np.asarray(bias, dtype=np.float32)

    # host-side routing + layout prep
    w_all = ktab[cls_idx].astype(BF16)                      # [B, C, F]
    b_all = bias[cls_idx]                                   # [B, F]
    add_bias = bool(np.any(b_all))
    xt_all = np.ascontiguousarray(
        x.reshape(B, NPIX, C).transpose(0, 2, 1)            # [B, C, NPIX]
    ).astype(BF16)

    key = ("cc11", add_bias)
    if key not in _CACHE:
        _CACHE[key] = _build(add_bias)
    nc = _CACHE[key]

    in_maps = []
    for c in range(NCORES):
        sl = slice(c * SPC, (c + 1) * SPC)
        m = {
            "xt": np.ascontiguousarray(xt_all[sl]),
            "wt": np.ascontiguousarray(w_all[sl]),
        }
        if add_bias:
            m["bt"] = np.ascontiguousarray(
                np.broadcast_to(b_all[sl, None, :], (SPC, P, F))
            )
        in_maps.append(m)

    res = run_bass_kernel_spmd(nc, in_maps, list(range(NCORES)))
    _last_results = res

    out = np.concatenate([res.results[c]["out"] for c in range(NCORES)], axis=0)
    # un-tile [B, NPB, P, PT, F] -> [B, NPIX, F] (pixel = pb*PB + pt*P + pi)
    out = out.transpose(0, 1, 3, 2, 4).astype(np.float32)
    return out.reshape(B, H, W, F)
